# revision 7
# baseline (speedup 1.0000x reference)
"""DigitCaps forward kernel for 8 Trainium2 NeuronCores.

Math: the reference collapses to
    s[b, cd] = (1/P) * sum_{p,e} x[b, p, e] * W[0, p, c, d, e]   (cd = c*16+d)
    v = s*|s| / (1 + s^2)                                        (elementwise squash)
    out = v.reshape(BS, C, D, 1)

i.e. one (512, 9216) @ (9216, 160) matmul + tiny elementwise epilogue.

Sharding: 8 cores = 4 batch-groups (128 rows) x 2 output-column halves (80 cols).
Each core reads its x slice (4.72 MB) + its W half (2.95 MB); no collectives.

Device layout: one input tensor per core, K-major, with each 128-deep k-tile
holding [x_tile (128x128) | w_tile (128x80)] side by side. One DMA per chunk
of k-tiles (single sem wait per dependent matmul — TRN2 instructions carry at
most one wait), 72 accumulating fp32 matmuls into one PSUM tile, all-DVE
squash epilogue, one small output DMA.
"""

import numpy as np

BS, P, C, D, E = 512, 1152, 10, 16, 8
K = P * E            # 9216 contraction
CD = C * D           # 160 output cols
KT = 128             # contraction per matmul tile
NKT = K // KT        # 72 k-tiles
NCORES = 8
BG = 4               # batch groups
MB = BS // BG        # 128 rows per group
NH = 2               # cd halves
NHW = CD // NH       # 80 cols per half
COLS = MB + NHW      # 208 cols per k-tile block
ALPHA = 1.0 / P

# DMA chunk sizes in k-tiles. 7 input DMAs + 1 output DMA = 8 total, one per
# HWDGE sem lane — no lane reuse, so no instruction needs a second sem wait
# (TRN2 instructions carry at most one).
CHUNKS = [11, 11, 10, 10, 10, 10, 10]
assert sum(CHUNKS) == NKT

TRACE = False        # set by test.py to profile
LAST_RESULT = {}     # exec_time_ns etc. for test.py

_CACHED_NC = None


def _build_kernel():
    import concourse.bass as bass
    import concourse.mybir as mybir
    import concourse.tile as tile

    f32 = mybir.dt.float32
    nc = bass.Bass()
    xw_d = nc.dram_tensor("xw", [KT, NKT * COLS], f32, kind="ExternalInput")
    o_d = nc.dram_tensor("o", [MB, NHW], f32, kind="ExternalOutput")

    with tile.TileContext(nc) as tc:
        with (
            tc.tile_pool(name="xwp", bufs=len(CHUNKS)) as xwp,
            tc.tile_pool(name="ep", bufs=1) as ep,
            tc.tile_pool(name="pp", bufs=1, space="PSUM") as pp,
        ):
            bufs = []
            t0 = 0
            for tpg in CHUNKS:
                xwg = xwp.tile([KT, tpg * COLS], f32, tag="xw")
                nc.sync.dma_start(
                    out=xwg[:], in_=xw_d[:, t0 * COLS:(t0 + tpg) * COLS]
                )
                bufs.append((xwg, t0, tpg))
                t0 += tpg

            ps = pp.tile([MB, NHW], f32)
            for xwg, t0, tpg in bufs:
                for j in range(tpg):
                    t = t0 + j
                    nc.tensor.matmul(
                        ps[:],
                        xwg[:, j * COLS:j * COLS + MB],
                        xwg[:, j * COLS + MB:(j + 1) * COLS],
                        start=(t == 0),
                        stop=(t == NKT - 1),
                    )

            # epilogue: s = ps*ALPHA; v = s*|s| / (1 + s^2)
            s = ep.tile([MB, NHW], f32)
            ng = ep.tile([MB, NHW], f32)
            a = ep.tile([MB, NHW], f32)
            n = ep.tile([MB, NHW], f32)
            q = ep.tile([MB, NHW], f32)
            d2 = ep.tile([MB, NHW], f32)
            r = ep.tile([MB, NHW], f32)
            v = ep.tile([MB, NHW], f32)
            nc.vector.tensor_scalar_mul(s[:], ps[:], ALPHA)
            nc.vector.tensor_scalar_mul(ng[:], ps[:], -ALPHA)
            nc.vector.tensor_tensor(a[:], s[:], ng[:], mybir.AluOpType.max)
            nc.vector.tensor_mul(n[:], s[:], a[:])
            nc.vector.tensor_mul(q[:], s[:], s[:])
            nc.vector.tensor_scalar_add(d2[:], q[:], 1.0)
            nc.vector.reciprocal(r[:], d2[:])
            nc.vector.tensor_mul(v[:], n[:], r[:])
            nc.sync.dma_start(out=o_d[:], in_=v[:])
    _split_multi_waits(nc)
    return nc


def _split_multi_waits(nc):
    """TRN2 instructions carry at most one semaphore wait; walrus rejects
    more. Tile's auto-emitted kernel-tail Drain waits on every engine/DMA
    sem. Split extra waits into standalone single-wait EventSemaphore
    instructions placed just before the owner, on the same engine."""
    import concourse.mybir as mybir

    for f in nc.m.functions:
        for blk in f.blocks:
            out = []
            changed = False
            for inst in blk.instructions:
                si = inst.sync_info
                waits = list(si.on_wait) if si and si.on_wait else []
                if len(waits) > 1:
                    changed = True
                    for k, w in enumerate(waits[:-1]):
                        out.append(mybir.InstEventSemaphore(
                            name=f"{inst.name}-sw{k}",
                            engine=inst.engine,
                            ins=[],
                            outs=[],
                            sync_info=mybir.SyncInfo(on_wait=[w], on_update=[]),
                        ))
                    inst.sync_info = mybir.SyncInfo(
                        on_wait=[waits[-1]],
                        on_update=list(si.on_update) if si.on_update else [],
                    )
                out.append(inst)
            if changed:
                blk.instructions = out


def _prep_inputs(x, W):
    """Build the per-core [k, t, (x|w)] interleaved operand arrays."""
    xr = np.ascontiguousarray(x, dtype=np.float32).reshape(BS, K)
    xgs = []
    for g in range(BG):
        xg = xr[g * MB:(g + 1) * MB, :].T.reshape(NKT, KT, MB)  # (t, k, b)
        xgs.append(np.transpose(xg, (1, 0, 2)))                  # (k, t, b)
    Wf = np.ascontiguousarray(
        np.asarray(W, dtype=np.float32)[0].transpose(0, 3, 1, 2)
    ).reshape(K, CD)
    whs = []
    for h in range(NH):
        wh = Wf[:, h * NHW:(h + 1) * NHW].reshape(NKT, KT, NHW)  # (t, k, n)
        whs.append(np.transpose(wh, (1, 0, 2)))                  # (k, t, n)
    maps = []
    for i in range(NCORES):
        g, h = i % BG, i // BG
        xw = np.concatenate([xgs[g], whs[h]], axis=2)            # (k, t, 208)
        maps.append({"xw": np.ascontiguousarray(xw).reshape(KT, NKT * COLS)})
    return maps


def kernel(x, W):
    global _CACHED_NC, LAST_RESULT
    from concourse.bass_utils import run_bass_kernel_spmd

    x = np.asarray(x, dtype=np.float32)
    W = np.asarray(W, dtype=np.float32)
    assert x.shape == (BS, P, E), x.shape
    assert W.shape == (1, P, C, D, E), W.shape

    if _CACHED_NC is None:
        _CACHED_NC = _build_kernel()
    nc = _CACHED_NC

    in_maps = _prep_inputs(x, W)
    res = run_bass_kernel_spmd(nc, in_maps, core_ids=list(range(NCORES)), trace=TRACE)
    LAST_RESULT = {"exec_time_ns": res.exec_time_ns,
                   "mean_exec_time_ns": res.mean_exec_time_ns,
                   "trace": res.instructions_and_trace}

    out = np.empty((BS, CD), dtype=np.float32)
    for i in range(NCORES):
        g, h = i % BG, i // BG
        out[g * MB:(g + 1) * MB, h * NHW:(h + 1) * NHW] = res.results[i]["o"]
    return out.reshape(BS, C, D, 1)
